# revision 1
# baseline (speedup 1.0000x reference)
"""Trainium2 Bass kernel for nn_BidPrefix (segment_reduce).

Problem: inputs [B=500000, 302] f32 rows = [rates[0:300], market_price, bid].
  cp1[k] = prod(rates[:k])  (exclusive prefix products, cp1[0] = 1)
  survival  = cp1[bid]
  rate_last = cp1[mp] - cp1[mp+1]

Kernel strategy (pure data parallel over 8 NeuronCores, batch sharded):
  Per 128-row tile on each core:
    - one DMA loads the [128, 302] tile into SBUF;
    - DVE tensor_tensor_scan computes the exact sequential f32 cumprod
      into cp1[:, 1:301] (cp1[:, 0] preset to 1.0 once per physical
      buffer) — identical rounding to the reference's jnp.cumprod;
    - three DVE scalar_tensor_tensor instructions perform exact per-row
      gathers: accum_out = sum((iota == idx) * cp1) = cp1[idx]. The
      cp1[mp+1] gather uses a shifted cp1 view so no idx arithmetic is
      needed. Gather results land in per-tile columns of persistent
      [128, ntiles] accumulators.
  Tail: rate_last = A1 - A2 in one wide subtract, then two DMAs store
  the accumulators to DRAM ([ntiles*128, 1] interleaved layout).

The whole kernel is DVE-bound (4 full-width DVE passes per tile); DMA,
ACT, PE, GPSIMD stay under its shadow.
"""

import numpy as np

SEQ = 300
W = SEQ + 2  # input columns
CP = SEQ + 1  # cumprod-with-leading-one columns
B = 500000
N_CORES = 8
ROWS_PER_CORE = 62592  # 489 tiles of 128 rows; 8*62592 = 500736 >= B
assert ROWS_PER_CORE % 128 == 0 and N_CORES * ROWS_PER_CORE >= B

_CACHE = {}


def _split_multi_waits(nc, max_waits=1):
    """Walrus in this container rejects instructions with >1 sync-wait.

    Hoist extra waits onto single-wait NOPs inserted right before the
    offending instruction on the same engine (same-queue program order
    preserves semantics).
    """
    import concourse.mybir as mybir

    ctr = 0
    for fn in nc.m.functions:
        for bb in fn.blocks:
            il = bb.instructions
            i = 0
            while i < len(il):
                ins = il[i]
                si = ins.sync_info
                if si is not None and si.on_wait and len(si.on_wait) > max_waits:
                    waits = list(si.on_wait)
                    pos = i
                    for w in waits[max_waits:]:
                        ctr += 1
                        nop = mybir.InstNoOp(
                            name=f"I-splitwait-{ctr}",
                            engine=ins.engine,
                            sync_info=mybir.SyncInfo(on_wait=[w], on_update=[]),
                        )
                        il.insert(pos, nop)
                        pos += 1
                        i += 1
                    si.on_wait = waits[:max_waits]
                i += 1


def _build_nc(rows=ROWS_PER_CORE, n_cp_bufs=4, in_bufs=4, trash_bufs=4):
    import concourse.bass as bass
    import concourse.tile as tile
    from concourse import mybir

    F32 = mybir.dt.float32
    I32 = mybir.dt.int32
    assert rows % 128 == 0
    ntiles = rows // 128

    nc = bass.Bass("TRN2")
    x = nc.dram_tensor("inputs", [rows, W], F32, kind="ExternalInput")
    out_s = nc.dram_tensor("surv", [rows, 1], F32, kind="ExternalOutput")
    out_r = nc.dram_tensor("ratelast", [rows, 1], F32, kind="ExternalOutput")

    x_t = x.rearrange("(t p) c -> t p c", p=128)
    out_s_t = out_s.rearrange("(t p) c -> p t c", p=128)
    out_r_t = out_r.rearrange("(t p) c -> p t c", p=128)

    with tile.TileContext(nc) as tc:
        with (
            tc.tile_pool(name="inp", bufs=in_bufs) as inp_pool,
            tc.tile_pool(name="trash", bufs=trash_bufs) as trash_pool,
            tc.tile_pool(name="persist", bufs=1) as persist,
        ):
            S = persist.tile([128, ntiles], F32, tag="acc_s")
            A1 = persist.tile([128, ntiles], F32, tag="acc_a1")
            A2 = persist.tile([128, ntiles], F32, tag="acc_a2")

            iota_i = persist.tile([128, CP], I32, tag="iota_i")
            nc.gpsimd.iota(iota_i[:, :], [[1, CP]], channel_multiplier=0)
            iota_f = persist.tile([128, CP], F32, tag="iota_f")
            nc.vector.tensor_copy(iota_f[:, :], iota_i[:, :])

            cp1_bufs = []
            for i in range(n_cp_bufs):
                t = persist.tile([128, CP], F32, tag=f"cp1_{i}")
                nc.gpsimd.memset(t[:, 0:1], 1.0)
                cp1_bufs.append(t)

            for i in range(ntiles):
                xt = inp_pool.tile([128, W], F32, tag="xt")
                nc.sync.dma_start(out=xt[:, :], in_=x_t[i, :, :])

                rates = xt[:, 0:SEQ]
                mp = xt[:, SEQ : SEQ + 1]
                bid = xt[:, SEQ + 1 : SEQ + 2]

                cp1 = cp1_bufs[i % n_cp_bufs]
                nc.vector.tensor_tensor_scan(
                    out=cp1[:, 1:CP],
                    data0=rates,
                    data1=rates,
                    initial=1.0,
                    op0=mybir.AluOpType.mult,
                    op1=mybir.AluOpType.bypass,
                )

                for idx_ap, data_ap, iota_ap, acc in (
                    (bid, cp1[:, :], iota_f[:, :], S[:, i : i + 1]),
                    (mp, cp1[:, :], iota_f[:, :], A1[:, i : i + 1]),
                    (mp, cp1[:, 1:CP], iota_f[:, 0 : CP - 1], A2[:, i : i + 1]),
                ):
                    tr = trash_pool.tile([128, CP], F32, tag="tr")
                    nc.vector.scalar_tensor_tensor(
                        out=tr[:, 0 : iota_ap.shape[1]],
                        in0=iota_ap,
                        scalar=idx_ap,
                        in1=data_ap,
                        op0=mybir.AluOpType.is_equal,
                        op1=mybir.AluOpType.mult,
                        accum_out=acc,
                    )

            nc.vector.tensor_sub(A1[:, :], A1[:, :], A2[:, :])
            nc.sync.dma_start(out=out_s_t[:, :, 0], in_=S[:, :])
            nc.sync.dma_start(out=out_r_t[:, :, 0], in_=A1[:, :])

    _split_multi_waits(nc)
    return nc


def _get_nc():
    if "nc" not in _CACHE:
        _CACHE["nc"] = _build_nc()
    return _CACHE["nc"]


def _shard_inputs(inputs):
    total = N_CORES * ROWS_PER_CORE
    padded = np.empty((total, W), dtype=np.float32)
    padded[: inputs.shape[0]] = inputs
    if total > inputs.shape[0]:
        padded[inputs.shape[0] :, :SEQ] = 1.0
        padded[inputs.shape[0] :, SEQ:] = 0.0
    return [
        padded[c * ROWS_PER_CORE : (c + 1) * ROWS_PER_CORE] for c in range(N_CORES)
    ]


def kernel(inputs: np.ndarray):
    from concourse.bass_utils import run_bass_kernel_spmd

    inputs = np.ascontiguousarray(inputs, dtype=np.float32)
    assert inputs.shape == (B, W), inputs.shape

    nc = _get_nc()
    shards = _shard_inputs(inputs)
    res = run_bass_kernel_spmd(
        nc,
        [{"inputs": s} for s in shards],
        core_ids=list(range(N_CORES)),
    )
    surv = np.concatenate([r["surv"] for r in res.results], axis=0)[:B]
    rl = np.concatenate([r["ratelast"] for r in res.results], axis=0)[:B]
    return surv, rl



# revision 9
# speedup vs baseline: 1.4217x; 1.4217x over previous
"""Trainium2 Bass kernel for nn_BidPrefix (segment_reduce).

Problem: inputs [B=500000, 302] f32 rows = [rates[0:300], market_price, bid].
  cp1[k] = prod(rates[:k])  (exclusive prefix products, cp1[0] = 1)
  survival  = cp1[bid]
  rate_last = cp1[mp] - cp1[mp+1]

Strategy (pure data parallel over 8 NeuronCores, batch sharded):
  R=8 row-blocks per partition: each 128-partition tile covers 1024 rows,
  loaded as one DMA of [128, 8*302] (8 contiguous 1208B runs/partition).

  Per tile:
   - DVE computes ONE affine prefix scan over all 8 blocks at once:
       state = state*d0[t] + d1[t]
     where d0 is the tile (bid columns zeroed by ACT so the state dies at
     each block boundary) and d1 is a constant one-hot (1.0 at each bid
     column) that restarts the state at 1.0 for the next block. The scan
     output is exactly the 8 blocks' exclusive cumprods, concatenated,
     with cp1-block k at buf cols [302k, 302k+300].
   - Small DVE ops build per-row gather indices (mp, mp+1, bid) + 302k
     block offsets, cast to int16.
   - GPSIMD ap_gather (sole Pool op; ap_gather ucode library loaded once)
     pulls all 3 gathers for 16 rows x 8 blocks per partition group in a
     single instruction: out[p, s*16+l] = buf[p, idx[16g+l, s]].
   - Row p only owns slots with l == p%16: one DVE mask-multiply against
     a constant 0/1 mask + one segmented tensor_reduce extracts the
     [128, 8, 3] results per tile into persistent accumulators.
  Tail: rate_last = A_mp - A_mp1 in one wide subtract; outputs stored as
  [128, T*R] per core, un-interleaved on the host (a.T.reshape).

DVE does ~466ns/128rows (scan-dominated), matching the DMA roofline of
466ns/128rows; ACT/GPSIMD stay under it. All constants (reset one-hot,
mask, offsets) are tiny ExternalInputs so GPSIMD never reloads ucode.
"""

import numpy as np

SEQ = 300
W = SEQ + 2  # 302 input columns per row
B = 500000
N_CORES = 8
R = 8  # row-blocks per partition
TILE_ROWS = 128 * R  # 1024
T = 62  # tiles per core
ROWS_PER_CORE = T * TILE_ROWS  # 63488; 8*63488 = 507904 >= B
WIDE = R * W  # 2416
SCAN_W = (R - 1) * W + SEQ  # 2414: cols 0..2413 feed the scan
NE = SCAN_W + 1  # 2415 gather num_elems (buf cols 0..2414)
NSLOT = 3 * R  # 24 index slots per partition
NIDX = 16 * NSLOT  # 384 gathered values per partition
OUT_COLS = T * R  # 496

_CACHE = {}


def _split_multi_waits(nc, max_waits=1):
    """Walrus in this container rejects instructions with >1 sync-wait.

    Hoist extra waits onto single-wait NOPs inserted right before the
    offending instruction on the same engine (same-queue program order
    preserves semantics).
    """
    import concourse.mybir as mybir

    ctr = 0
    for fn in nc.m.functions:
        for bb in fn.blocks:
            il = bb.instructions
            i = 0
            while i < len(il):
                ins = il[i]
                si = ins.sync_info
                if si is not None and si.on_wait and len(si.on_wait) > max_waits:
                    waits = list(si.on_wait)
                    pos = i
                    for w in waits[max_waits:]:
                        ctr += 1
                        nop = mybir.InstNoOp(
                            name=f"I-splitwait-{ctr}",
                            engine=ins.engine,
                            sync_info=mybir.SyncInfo(on_wait=[w], on_update=[]),
                        )
                        il.insert(pos, nop)
                        pos += 1
                        i += 1
                    si.on_wait = waits[:max_waits]
                i += 1


def make_consts():
    """Host-built constant tensors shipped to every core."""
    wm = np.zeros((128, NIDX), np.float32)
    for p in range(128):
        wm[p, np.arange(NSLOT) * 16 + p % 16] = 1.0
    rst = np.zeros((128, SCAN_W), np.float32)
    for k in range(R - 1):
        rst[:, k * W + SEQ + 1] = 1.0
    offa = np.zeros((128, R, 2), np.float32)
    offb = np.zeros((128, R), np.float32)
    for k in range(R):
        offa[:, k, :] = k * W
        offb[:, k] = k * W + 1
    return {"wm": wm, "rst": rst, "offa": offa, "offb": offb}


def _build_nc(in_bufs=4, g_bufs=3, repeat=1):
    import concourse.bass as bass
    import concourse.tile as tile
    from concourse import mybir, library_config

    F32 = mybir.dt.float32
    I16 = mybir.dt.int16

    nc = bass.Bass("TRN2")
    x = nc.dram_tensor("inputs", [ROWS_PER_CORE, W], F32, kind="ExternalInput")
    c_wm = nc.dram_tensor("wm", [128, NIDX], F32, kind="ExternalInput")
    c_rst = nc.dram_tensor("rst", [128, SCAN_W], F32, kind="ExternalInput")
    c_offa = nc.dram_tensor("offa", [128, R, 2], F32, kind="ExternalInput")
    c_offb = nc.dram_tensor("offb", [128, R], F32, kind="ExternalInput")
    out_s = nc.dram_tensor("surv", [128, OUT_COLS], F32, kind="ExternalOutput")
    out_r = nc.dram_tensor("ratelast", [128, OUT_COLS], F32, kind="ExternalOutput")

    x_t = x.rearrange("(t r p) c -> t p r c", p=128, r=R)

    with tile.TileContext(nc) as tc:
        with (
            tc.tile_pool(name="inp", bufs=in_bufs) as inp_pool,
            tc.tile_pool(name="gat", bufs=g_bufs) as gat_pool,
            tc.tile_pool(name="idx", bufs=3) as idx_pool,
            tc.tile_pool(name="persist", bufs=1) as persist,
        ):
            WM = persist.tile([128, NIDX], F32, tag="wm")
            RST = persist.tile([128, SCAN_W], F32, tag="rst")
            OFFA = persist.tile([128, R, 2], F32, tag="offa")
            OFFB = persist.tile([128, R], F32, tag="offb")
            nc.sync.dma_start(out=WM[:, :], in_=c_wm[:, :])
            nc.sync.dma_start(out=RST[:, :], in_=c_rst[:, :])
            nc.sync.dma_start(out=OFFA[:, :, :], in_=c_offa[:, :, :])
            nc.sync.dma_start(out=OFFB[:, :], in_=c_offb[:, :])

            nc.gpsimd.load_library(library_config.ap_gather)

            ACC = persist.tile([128, T * R, 3], F32, tag="acc")
            RL = persist.tile([128, OUT_COLS], F32, tag="rl")

            # rotating cp1 buffers; col 0 preset to 1.0 once
            n_cp = 3
            cp_bufs = []
            for j in range(n_cp):
                t = persist.tile([128, NE], F32, tag=f"cp1_{j}")
                nc.vector.memset(t[:, 0:1], 1.0)
                cp_bufs.append(t)

            for i in [i for _ in range(repeat) for i in range(T)]:
                xt = inp_pool.tile([128, WIDE], F32, tag="xt")
                xt3 = xt[:, :].rearrange("p (r c) -> p r c", r=R)
                nc.sync.dma_start(out=xt3, in_=x_t[i, :, :, :])

                # per-row gather indices (mp, mp+1, bid) + block offsets
                idxf = idx_pool.tile([128, R, 3], F32, tag="idxf")
                nc.vector.scalar_tensor_tensor(
                    out=idxf[:, :, 0:3:2],
                    in0=xt3[:, :, SEQ : SEQ + 2],
                    scalar=0.0,
                    in1=OFFA[:, :, :],
                    op0=mybir.AluOpType.add,
                    op1=mybir.AluOpType.add,
                )
                nc.vector.scalar_tensor_tensor(
                    out=idxf[:, :, 1:2],
                    in0=xt3[:, :, SEQ : SEQ + 1],
                    scalar=0.0,
                    in1=OFFB[:, :, None],
                    op0=mybir.AluOpType.add,
                    op1=mybir.AluOpType.add,
                )
                idx16 = idx_pool.tile([128, NSLOT], I16, tag="idx16")
                nc.scalar.copy(
                    idx16[:, :], idxf[:, :, :].rearrange("p a b -> p (a b)")
                )

                # kill the scan state at block boundaries (bid columns)
                nc.scalar.mul(
                    xt[:, SEQ + 1 : SCAN_W : W], xt[:, SEQ + 1 : SCAN_W : W], 0.0
                )

                buf = cp_bufs[i % n_cp]
                nc.vector.tensor_tensor_scan(
                    out=buf[:, 1:NE],
                    data0=xt[:, 0:SCAN_W],
                    data1=RST[:, :],
                    initial=1.0,
                    op0=mybir.AluOpType.mult,
                    op1=mybir.AluOpType.add,
                )

                G = gat_pool.tile([128, NIDX], F32, tag="g")
                nc.gpsimd.ap_gather(
                    G[:, :], buf[:, :], idx16[:, :],
                    channels=128, num_elems=NE, d=1, num_idxs=NIDX,
                )

                GM = gat_pool.tile([128, NIDX], F32, tag="gm")
                nc.vector.tensor_tensor(
                    GM[:, :], G[:, :], WM[:, :], mybir.AluOpType.mult
                )
                nc.vector.tensor_reduce(
                    out=ACC[:, i * R : (i + 1) * R, :],
                    in_=GM[:, :].rearrange("p (s l) -> p s l", l=16),
                    op=mybir.AluOpType.add,
                    axis=mybir.AxisListType.X,
                )

            nc.vector.tensor_tensor(
                RL[:, :], ACC[:, :, 0], ACC[:, :, 1], mybir.AluOpType.subtract
            )
            nc.sync.dma_start(out=out_s[:, :], in_=ACC[:, :, 2])
            nc.sync.dma_start(out=out_r[:, :], in_=RL[:, :])

    _split_multi_waits(nc)
    # Raw Bass skips Bacc's codegen pass that fills in .instr bytes for
    # extended-ISA instructions (ap_gather, library load); without it the
    # NEFF compiler fails with "ISA wrong length".
    from concourse.library_overlay import lower_extended_insts

    lower_extended_insts(nc)
    return nc


def _get_nc():
    if "nc" not in _CACHE:
        _CACHE["nc"] = _build_nc()
    return _CACHE["nc"]


def _shard_inputs(inputs):
    total = N_CORES * ROWS_PER_CORE
    padded = np.empty((total, W), dtype=np.float32)
    padded[: inputs.shape[0]] = inputs
    if total > inputs.shape[0]:
        padded[inputs.shape[0] :, :SEQ] = 1.0
        padded[inputs.shape[0] :, SEQ:] = 0.0
    return [
        padded[c * ROWS_PER_CORE : (c + 1) * ROWS_PER_CORE] for c in range(N_CORES)
    ]


def kernel(inputs: np.ndarray):
    from concourse.bass_utils import run_bass_kernel_spmd

    inputs = np.ascontiguousarray(inputs, dtype=np.float32)
    assert inputs.shape == (B, W), inputs.shape

    nc = _get_nc()
    shards = _shard_inputs(inputs)
    consts = make_consts()
    res = run_bass_kernel_spmd(
        nc,
        [{"inputs": s, **consts} for s in shards],
        core_ids=list(range(N_CORES)),
    )
    surv = np.concatenate(
        [r["surv"].T.reshape(-1, 1) for r in res.results], axis=0
    )[:B]
    rl = np.concatenate(
        [r["ratelast"].T.reshape(-1, 1) for r in res.results], axis=0
    )[:B]
    return surv, rl
